# revision 22
# baseline (speedup 1.0000x reference)
"""Bass/Trainium2 kernel for GQA transformer block (nn_GQA_84353157694016).

Reference computation (B=2, S=2048, E=4096, H=32 q-heads, KVH=8 kv-heads, D=128):
    qkv = x @ wqkv.T                  -> split into q/k/v per GQA group
    q,k = rope_interleaved(q), rope_interleaved(k)
    out = softmax(causal(q k^T / sqrt(D))) @ v @ wo.T

Sharding (8 cores): data-parallel over batch (2 groups of 4 cores) x
tensor-parallel over heads (4 cores: 2 kv groups / 8 q heads each).  wo is
sharded on its input dim; the partial outputs are summed on the host
(the unshard step of the reduce).

Layout strategy: everything on-chip is computed in "transposed" (feature x
sequence) orientation so the TensorE contraction dim always lands on
partitions with zero on-chip transposes (except v, which is PE-transposed).
Softmax is computed without max-subtraction (scores are bounded ~ +-10 for
this problem size/scale), with the row-sum obtained by an extra ones-matmul
in the same PSUM-accumulation pass as attn @ v.
"""

import os
import sys

import numpy as np
import ml_dtypes

for _p in ("/opt/trn_rl_repo",):
    if _p not in sys.path and os.path.isdir(_p):
        sys.path.append(_p)

import concourse.bass as bass
import concourse.tile as tile
from concourse import bacc, mybir
from concourse.bass_utils import run_bass_kernel_spmd
from concourse.masks import make_identity


def _install_ntff_hook():
    """bass_utils' trace path imports antenv.axon_hooks, which the agent image
    lacks; synthesize it (backed by trn_boot's ctypes NTFF driver) so
    trace=True / BASS_TRACE=1 works instead of crashing."""
    try:
        import antenv.axon_hooks  # noqa: F401
        return
    except ImportError:
        pass
    try:
        import types
        import antenv
        mod = types.ModuleType("antenv.axon_hooks")
        mod._hook = None
        mod.set_axon_ntff_profile_hook = lambda h: setattr(mod, "_hook", h)
        mod.get_axon_ntff_profile_hook = lambda: mod._hook
        sys.modules["antenv.axon_hooks"] = mod
        antenv.axon_hooks = mod
        from trn_agent_boot.trn_boot import _ntff_profile_via_ctypes
        so = "/opt/axon/libaxon_pjrt.so"
        if os.path.exists(so):
            mod._hook = _ntff_profile_via_ctypes(so)
    except Exception:
        pass


_install_ntff_hook()

# problem constants
B, S, E = 2, 2048, 4096
H, KVH, D = 32, 8, 128
QPK = H // KVH                    # 4 q heads per kv group
ROPE_BASE = 10000.0

NCORES = 8
TP = 4                            # tensor-parallel width (heads)
DP = 2                            # data-parallel width (batch)

SC = 4                            # number of s-chunks == q strips
CW = S // SC                      # 512 chunk width
NJT = (E + 2 * KVH * D) // TP // 128   # 12 qkv row-tiles per core
NET = E // 128                    # 32 contraction tiles for qkv proj
GPC = KVH // TP                   # 2 kv groups per core
HPC = H // TP                     # 8 q heads per core
FT = HPC * D // 128               # 8 local ctx feature tiles
ECN = E // CW                     # 8 output e-chunks

f32 = mybir.dt.float32
f32r = mybir.dt.float32r
bf16 = mybir.dt.bfloat16
np_bf16 = ml_dtypes.bfloat16

_built = {}


def _build_nc():
    nc = bacc.Bacc("TRN2", target_bir_lowering=False)

    xt_d = nc.dram_tensor("xt", [SC, 128, NET, CW], bf16, kind="ExternalInput")
    wq_d = nc.dram_tensor("wq", [NJT, 128, NET, 128], bf16, kind="ExternalInput")
    wo_d = nc.dram_tensor("wo", [ECN, 128, FT, CW], bf16, kind="ExternalInput")
    cq_d = nc.dram_tensor("cq", [128, S], f32, kind="ExternalInput")
    sq_d = nc.dram_tensor("sq", [128, S], f32, kind="ExternalInput")
    ck_d = nc.dram_tensor("ck", [128, S], f32, kind="ExternalInput")
    sk_d = nc.dram_tensor("sk", [128, S], f32, kind="ExternalInput")
    mk_d = nc.dram_tensor("mk", [128, SC, CW], bf16, kind="ExternalInput")
    out_d = nc.dram_tensor("out", [S // 128, ECN, 128, CW], f32, kind="ExternalOutput")

    with tile.TileContext(nc) as tc:
        with (
            tc.tile_pool(name="const", bufs=1) as constp,
            tc.tile_pool(name="tab", bufs=2) as tabp,
            tc.tile_pool(name="xt", bufs=3) as xtp,
            tc.tile_pool(name="wq", bufs=2) as wqp,
            tc.tile_pool(name="st", bufs=8) as stp,
            tc.tile_pool(name="rt", bufs=2) as rtp,
            tc.tile_pool(name="q", bufs=2) as qp,
            tc.tile_pool(name="kv", bufs=1) as kvp,
            tc.tile_pool(name="at", bufs=6) as atp,
            tc.tile_pool(name="ctx", bufs=2) as ctxp,
            tc.tile_pool(name="wop", bufs=4) as wop,
            tc.tile_pool(name="ob", bufs=3) as obp,
            tc.tile_pool(name="rc", bufs=2) as rcp,
            tc.tile_pool(name="pmm", bufs=2, space="PSUM") as pmm,
            tc.tile_pool(name="pqk", bufs=3, space="PSUM") as pqk,
            tc.tile_pool(name="pacc", bufs=3, space="PSUM") as pacc,
        ):
            def emit_wo_block(cs, ec, ctx_tiles):
                """Output-projection block: out[strip cs, ec] += ctx @ woT."""
                wo_sb = wop.tile([128, FT, CW], bf16, tag="wo", name="wo_sb")
                nc.gpsimd.dma_start(out=wo_sb[:, :FT // 2, :],
                                    in_=wo_d[ec, :, :FT // 2, :])
                nc.gpsimd.dma_start(out=wo_sb[:, FT // 2:, :],
                                    in_=wo_d[ec, :, FT // 2:, :])
                for sti in range(CW // 128):
                    ps = pacc.tile([128, CW], f32, tag="acc", name="wo_ps")
                    for ft in range(FT):
                        nc.tensor.matmul(
                            ps,
                            lhsT=ctx_tiles[:, ft, sti * 128:(sti + 1) * 128],
                            rhs=wo_sb[:, ft, :],
                            start=(ft == 0),
                            stop=(ft == FT - 1),
                        )
                    ob = obp.tile([128, CW], f32, tag="ob", name="ob")
                    nc.vector.tensor_copy(ob, ps)
                    nc.scalar.dma_start(
                        out=out_d[(CW // 128) * cs + sti, ec], in_=ob
                    )

            # constants
            ident = constp.tile([128, 128], f32, tag="ident")
            make_identity(nc, ident)
            ones_sb = constp.tile([128, 128], bf16, tag="ones")
            nc.vector.memset(ones_sb, 1.0)
            mk_sb = constp.tile([128, SC, CW], bf16, tag="mk")

            # persistent k (transposed) and v (natural) per kv group, bf16
            k_sb = [kvp.tile([128, S], bf16, tag=f"k{g}", name=f"k{g}")
                    for g in range(GPC)]
            v_sb = [kvp.tile([128, S // 128, 128], bf16, tag=f"v{g}", name=f"v{g}")
                    for g in range(GPC)]

            for c in range(SC):
                csl = slice(c * CW, (c + 1) * CW)
                # head-start: first two weight tiles ahead of the big x loads
                wq_pre = []
                for jt in range(2):
                    w_ = wqp.tile([128, NET, 128], bf16, tag="wq", name="wq_pre")
                    nc.sync.dma_start(out=w_, in_=wq_d[jt])
                    wq_pre.append(w_)
                # x^T chunk: [128, 32, 512] bf16 as two halves for prefetch
                xt_h = []
                for h2 in range(2):
                    xh = xtp.tile([128, NET // 2, CW], bf16, tag="xt")
                    nc.sync.dma_start(
                        out=xh, in_=xt_d[c, :, h2 * (NET // 2):(h2 + 1) * (NET // 2), :]
                    )
                    xt_h.append(xh)

                # per-chunk rope table slices (needed only at RoPE time)
                cq_sb = tabp.tile([128, CW], f32, tag="cq")
                sq_sb = tabp.tile([128, CW], f32, tag="sq")
                ck_sb = tabp.tile([128, CW], f32, tag="ck")
                sk_sb = tabp.tile([128, CW], f32, tag="sk")
                nc.gpsimd.dma_start(out=cq_sb, in_=cq_d[:, csl])
                nc.gpsimd.dma_start(out=sq_sb, in_=sq_d[:, csl])
                nc.gpsimd.dma_start(out=ck_sb, in_=ck_d[:, csl])
                nc.gpsimd.dma_start(out=sk_sb, in_=sk_d[:, csl])
                if c == 0:
                    nc.gpsimd.dma_start(out=mk_sb, in_=mk_d[:])

                # ---- fused QKV projection + RoPE + v transpose, per kv group ----
                q_sb = qp.tile([128, HPC, CW], bf16, tag="q")
                for g in range(GPC):
                    stage = []
                    for sub in range(6):     # 4 q tiles, 1 k tile, 1 v tile
                        jt = 6 * g + sub
                        if jt < 2:
                            wq_sb = wq_pre[jt]
                        else:
                            wq_sb = wqp.tile([128, NET, 128], bf16, tag="wq")
                            nc.sync.dma_start(out=wq_sb, in_=wq_d[jt])
                        ps = pmm.tile([128, CW], f32, tag="mm")
                        for et in range(NET):
                            nc.tensor.matmul(
                                ps,
                                lhsT=wq_sb[:, et, :],
                                rhs=xt_h[et // (NET // 2)][:, et % (NET // 2), :],
                                start=(et == 0),
                                stop=(et == NET - 1),
                            )
                        st = stp.tile([128, CW], f32, tag="st")
                        nc.scalar.copy(st, ps)
                        stage.append(st)
                    for sub in range(QPK + 1):  # RoPE on 4 q tiles + 1 k tile
                        stq = stage[sub]
                        is_q = sub < QPK
                        # interleaved pair-swap via partition-strided DMA
                        sw = rtp.tile([128, CW], f32, tag="sw")
                        nc.sync.dma_start(out=sw[0::2, :], in_=stq[1::2, :])
                        nc.sync.dma_start(out=sw[1::2, :], in_=stq[0::2, :])
                        tmp = rtp.tile([128, CW], f32, tag="rt")
                        nc.vector.tensor_mul(tmp, sw, sq_sb if is_q else sk_sb)
                        nc.vector.tensor_mul(stq, stq, cq_sb if is_q else ck_sb)
                        if is_q:
                            nc.vector.tensor_add(q_sb[:, QPK * g + sub, :], stq, tmp)
                        else:
                            nc.vector.tensor_add(k_sb[g][:, csl], stq, tmp)
                    stv = stage[5]
                    for u in range(CW // 128):
                        tp_ = pmm.tile([128, CW], f32, tag="mm")
                        nc.tensor.transpose(
                            tp_[:, :128], stv[:, u * 128:(u + 1) * 128], ident
                        )
                        nc.scalar.copy(
                            v_sb[g][:, (CW // 128) * c + u, :], tp_[:, :128]
                        )

                # ---- attention for q strip c (flash-style, no max) ----
                njt2 = (CW // 128) * (c + 1)     # causal: k tiles 0..4c+3
                ctx_sb = ctxp.tile([128, HPC, CW], bf16, tag="ctx")
                for g in range(GPC):
                    for hq in range(QPK):
                        h = QPK * g + hq
                        if c > 0:
                            # software pipeline: strip c-1's output projection
                            # block (ec = h) fills PE while ACT/DVE run softmax
                            emit_wo_block(c - 1, h, prev_ctx)
                        ctx_ps = pacc.tile([128, CW], f32, tag="acc")
                        sums_ps = pacc.tile([128, CW], f32, tag="acc")
                        at_acc = None
                        for j2 in range(njt2):
                            # diagonal k-tiles: trim the fully-masked columns
                            # from the QK matmul and exp; zero-fill that part
                            # of the attn tile so AV/sums stay full-width
                            diag = j2 >= njt2 - (CW // 128)
                            o = 128 * (j2 - (njt2 - (CW // 128))) if diag else 0
                            nw = CW - o
                            qk = pqk.tile([128, CW], f32, tag="qk")
                            nc.tensor.matmul(
                                qk[:, :nw],
                                lhsT=k_sb[g][:, j2 * 128:(j2 + 1) * 128],
                                rhs=q_sb[:, h, o:],
                                start=True, stop=True,
                            )
                            at = atp.tile([128, CW], bf16, tag="at")
                            if o:
                                nc.gpsimd.memset(at[:, :o], 0.0)
                            nc.scalar.activation(
                                at[:, o:], qk[:, :nw],
                                mybir.ActivationFunctionType.Exp
                            )
                            if diag:
                                nc.vector.tensor_mul(
                                    at[:, o:o + 128], at[:, o:o + 128],
                                    mk_sb[:, 0, :128],
                                )
                            first, last = j2 == 0, j2 == njt2 - 1
                            nc.tensor.matmul(
                                ctx_ps, lhsT=v_sb[g][:, j2, :], rhs=at,
                                start=first, stop=last,
                            )
                            # batch the row-sum matmul over groups of 4 attn
                            # tiles: accumulate on DVE (bf16), one ones-matmul
                            # per group instead of per tile
                            ph = j2 % 4
                            if ph == 0:
                                at_prev = at
                            elif ph == 1:
                                at_acc = atp.tile([128, CW], bf16, tag="ata",
                                                  name="at_acc", bufs=3)
                                nc.vector.tensor_add(at_acc, at_prev, at)
                            else:
                                nc.vector.tensor_add(at_acc, at_acc, at)
                            if ph == 3:
                                nc.tensor.matmul(
                                    sums_ps, lhsT=ones_sb, rhs=at_acc,
                                    start=(j2 == 3), stop=(j2 == njt2 - 1),
                                )
                        rc = rcp.tile([128, CW], f32, tag="rc")
                        nc.vector.reciprocal_approx_fast(out=rc, in_=sums_ps)
                        nc.vector.tensor_mul(ctx_sb[:, h, :], ctx_ps, rc)

                prev_ctx = ctx_sb

            # drain: output projection for the final strip
            for ec in range(ECN):
                emit_wo_block(SC - 1, ec, prev_ctx)
    nc.finalize()
    return nc


def _rope_tables(scale):
    inv = 1.0 / (ROPE_BASE ** (np.arange(0, D, 2, dtype=np.float64) / D))
    ang = np.arange(S, dtype=np.float64)[None, :] * inv[:, None]    # [D/2, S]
    C = np.empty((D, S), np.float32)
    Sx = np.empty((D, S), np.float32)
    C[0::2] = np.cos(ang)
    C[1::2] = np.cos(ang)
    Sx[0::2] = -np.sin(ang)
    Sx[1::2] = np.sin(ang)
    return (C * scale).astype(np.float32), (Sx * scale).astype(np.float32)


def _host_inputs(x, wqkv, wo):
    """Shard + retile inputs for the 8 cores. Core c = 4*db + t."""
    cq, sq = _rope_tables(D ** -0.5)
    ck, sk = _rope_tables(1.0)

    # causal mask tiles in scores^T layout: keep when jj + 128*r <= ii
    jj = np.arange(128)[:, None]
    ii = np.arange(CW)[None, :]
    mk = np.empty((128, SC, CW), np_bf16)
    for r in range(SC):
        mk[:, r, :] = (jj + 128 * r <= ii).astype(np_bf16)

    xts = []
    for db in range(DP):
        xT = np.ascontiguousarray(x[db].T)                 # [E, S]
        t = xT.reshape(NET, 128, SC, CW).transpose(2, 1, 0, 3)
        xts.append(np.ascontiguousarray(t.astype(np_bf16)))

    wqs, wos = [], []
    rows = (E + 2 * KVH * D) // TP
    for t in range(TP):
        wT = np.ascontiguousarray(wqkv[rows * t:rows * (t + 1)].T)   # [E, 1536]
        wq_t = wT.reshape(NET, 128, NJT, 128).transpose(2, 1, 0, 3)
        wqs.append(np.ascontiguousarray(wq_t.astype(np_bf16)))
        woT = np.ascontiguousarray(wo[:, 1024 * t:1024 * (t + 1)].T)  # [1024, E]
        wo_t = woT.reshape(FT, 128, ECN, CW).transpose(2, 1, 0, 3)
        wos.append(np.ascontiguousarray(wo_t.astype(np_bf16)))

    in_maps = []
    for c in range(NCORES):
        db, t = divmod(c, TP)
        in_maps.append({
            "xt": xts[db], "wq": wqs[t], "wo": wos[t],
            "cq": cq, "sq": sq, "ck": ck, "sk": sk,
            "mk": mk,
        })
    return in_maps


def kernel(x, wqkv, wo):
    x = np.asarray(x, np.float32)
    wqkv = np.asarray(wqkv, np.float32)
    wo = np.asarray(wo, np.float32)

    if "nc" not in _built:
        _built["nc"] = _build_nc()
    nc = _built["nc"]

    in_maps = _host_inputs(x, wqkv, wo)
    res = run_bass_kernel_spmd(nc, in_maps, core_ids=list(range(NCORES)))
    globals()["_last_results"] = res

    out = np.zeros((B, S, E), np.float32)
    for c in range(NCORES):
        db = c // TP
        o = res.results[c]["out"]                       # [16, 8, 128, 512]
        out[db] += o.transpose(0, 2, 1, 3).reshape(S, E)
    return out
